# revision 5
# baseline (speedup 1.0000x reference)
"""Trainium2 Bass kernel for nn_ClinicalEmbedding (EmbeddingBag-style ragged gather).

Semantics (matches reference.py):
  flat = codes.reshape(B, L); g = renorm(W[flat])  (max_norm=1.0)
  out[b, v] = 0                       for v <  V - nv[b]
            = g[b, v - (V-nv[b])]     for V-nv[b] <= v < V-1
            = sum_{j=nv-1}^{nv*C-1} g[b, j]   for v = V-1

Strategy (data-parallel over batch, W replicated, per the sharding hint):
  * Bag rows (v = V-1): instead of per-row indirect gathers (the SWDGE
    descriptor path costs ~4.4us per 128 rows on this part), each core
    STREAMS the whole table W once with large contiguous DMAs and
    accumulates  psum[slot, e] += map_chunk^T @ W_chunk  on the Tensor
    engine, where map = a * rsqrt(max(1, ||W_row||^2)) folds the count
    matrix a[row, slot] (host-built from codes) and the max_norm renorm
    (device-computed) into the matmul weights.  W is host-permuted into a
    chunk-major group layout so each group is one contiguous
    8KB-per-partition read; the stream alternates between the two HWDGE
    rings (sync/scalar engines).
  * Single rows (v < V-1): 13 single-column indirect gathers (1664 rows)
    on gpsimd, fully overlapped with the stream; renormalized and masked
    on DVE, then written with one affine DMA in a v-major layout.
  * All bag arithmetic is fp32 (bf16 terms would fail the max-rel-err
    gate via random-walk error over ~800 summed terms); the count matrix
    `a` ships as bf16 (small ints, exact).
"""

import os

import numpy as np
import ml_dtypes

import concourse.bacc as bacc
import concourse.bass as bass
import concourse.mybir as mybir
import concourse.tile as tile
from concourse.bass_utils import run_bass_kernel_spmd

P = 128          # SBUF partitions
N_CORES = 8

GC = 16          # W chunks per stream group
NG = 49          # stream groups; NG*GC*P = 100352 padded vocab rows
VPAD = NG * GC * P

LAST_RESULTS = None   # test harness reads profiling info from here


def _prepare(codes, nv, B, V, C, L, VOCAB, E):
    """Host-side index/count construction. Uses only codes/n_visits for
    structure; W itself is only layout-permuted (value-independent)."""
    B_LOC = B // N_CORES
    assert B_LOC == 32 and V == 50 and E == 128

    # device W is stored permuted (see permute_W); map an original row id to
    # its permuted position
    def perm_idx(w):
        g, rem = w // (GC * P), w % (GC * P)
        return g * (GC * P) + (rem % P) * GC + rem // P

    # singles stream position i = v*32 + s  (v in [0,52), s in [0,32)),
    # landing at gS partition p = i % 128, column j = i // 128:
    #   p = 32*(v % 4) + s,  j = v // 4
    NJ = 13
    idxS = np.zeros((N_CORES, P, NJ), np.int32)
    wS = np.zeros((N_CORES, P, NJ), np.float32)
    for k in range(N_CORES):
        for s in range(B_LOC):
            b = k * B_LOC + s
            n = int(nv[b])
            for v in range(V - 1):        # v = 0..48 are candidate single rows
                p = 32 * (v % 4) + s
                j = v // 4
                if v >= V - n:
                    idxS[k, p, j] = perm_idx(int(codes[b, v - (V - n)]))
                    wS[k, p, j] = 1.0

    # bag count matrix a[row, slot] then permute rows into the stream layout:
    # W_perm[g*2048 + p*GC + kk] = W_pad[g*2048 + kk*128 + p]
    A = np.zeros((N_CORES, VPAD, B_LOC), np.float32)
    for k in range(N_CORES):
        for s in range(B_LOC):
            b = k * B_LOC + s
            n = int(nv[b])
            bag = codes[b, n - 1 : n * C]
            np.add.at(A[k], (bag, np.full(len(bag), s)), 1.0)
    A = A.reshape(N_CORES, NG, GC, P, B_LOC).transpose(0, 1, 3, 2, 4)
    A = np.ascontiguousarray(A.reshape(N_CORES, VPAD, B_LOC))

    return dict(B_LOC=B_LOC, idxS=idxS, wS=wS, A=A)


def permute_W(W, VOCAB, E):
    """Pad W to VPAD rows and apply the chunk-major stream permutation."""
    Wp = np.zeros((VPAD, E), np.float32)
    Wp[:VOCAB] = W
    Wp = Wp.reshape(NG, GC, P, E).transpose(0, 2, 1, 3)
    return np.ascontiguousarray(Wp.reshape(VPAD, E))


def _build(V, E, B_LOC):
    f32 = mybir.dt.float32
    bf16 = mybir.dt.bfloat16
    i32 = mybir.dt.int32
    NJ = 13

    nc = bacc.Bacc("TRN2", num_devices=N_CORES, debug=False)
    W_d = nc.dram_tensor("W", [VPAD, E], f32, kind="ExternalInput")
    A_d = nc.dram_tensor("A", [VPAD, B_LOC], bf16, kind="ExternalInput")
    idxS_d = nc.dram_tensor("idxS", [P, NJ], i32, kind="ExternalInput")
    wS_d = nc.dram_tensor("wS", [P, NJ], f32, kind="ExternalInput")
    out_d = nc.dram_tensor("out", [B_LOC * V, E], f32, kind="ExternalOutput")

    W_v = W_d[:].rearrange("(g p k) e -> g p k e", p=P, k=GC)
    A_v = A_d[:].rearrange("(g p k) s -> g p k s", p=P, k=GC)

    with tile.TileContext(nc) as tc:
        with (
            tc.tile_pool(name="const", bufs=1) as cpool,
            tc.tile_pool(name="w", bufs=3) as wpool,
            tc.tile_pool(name="a", bufs=2) as apool,
            tc.tile_pool(name="sq", bufs=2) as sqpool,
            tc.tile_pool(name="sm", bufs=3) as smpool,
            tc.tile_pool(name="sing", bufs=1) as spool,
            tc.tile_pool(name="ps", bufs=1, space="PSUM") as pspool,
        ):
            idxS_t = cpool.tile_from(idxS_d[:])
            wS_t = cpool.tile_from(wS_d[:])

            psum = pspool.tile([B_LOC, E], f32)

            # zero bias tile written by DVE so ACT ops wait only on DVE
            zbias = cpool.tile([P, 1], f32, tag="zbias")
            nc.vector.memset(zbias[:], 0.0)

            # ---- singles gathers first: Pool desc-gen overlaps the stream ----
            gS = spool.tile([P, NJ * E], f32, tag="gS", bufs=1)
            for j in range(NJ):
                nc.gpsimd.indirect_dma_start(
                    out=gS[:, j * E : (j + 1) * E], out_offset=None, in_=W_d[:],
                    in_offset=bass.IndirectOffsetOnAxis(
                        ap=idxS_t[:, j : j + 1], axis=0
                    ),
                )

            # ---------------- bag pass: stream W ----------------
            for g in range(NG):
                engW = nc.scalar if g % 2 else nc.sync
                engA = nc.sync if g % 2 else nc.scalar
                Wg = wpool.tile([P, GC * E], f32, tag="Wg")
                engW.dma_start(
                    out=Wg[:].rearrange("p (k e) -> p k e", e=E), in_=W_v[g]
                )
                Ag = apool.tile([P, GC * B_LOC], bf16, tag="Ag")
                engA.dma_start(
                    out=Ag[:].rearrange("p (k s) -> p k s", s=B_LOC), in_=A_v[g]
                )
                sqg = sqpool.tile([P, GC * E], f32, tag="sqg")
                nc.scalar.activation(
                    sqg[:], Wg[:], mybir.ActivationFunctionType.Square,
                    bias=zbias[:],
                )
                n2 = smpool.tile([P, GC], f32, tag="n2")
                nc.vector.tensor_reduce(
                    n2[:], sqg[:].rearrange("p (k e) -> p k e", e=E),
                    axis=mybir.AxisListType.X, op=mybir.AluOpType.add,
                )
                nc.vector.tensor_scalar_max(n2[:], n2[:], 1.0)
                sq2 = smpool.tile([P, GC], f32, tag="sq2")
                nc.scalar.activation(
                    sq2[:], n2[:], mybir.ActivationFunctionType.Sqrt,
                    bias=zbias[:],
                )
                r = smpool.tile([P, GC], f32, tag="r")
                nc.vector.reciprocal(r[:], sq2[:])
                mapg = smpool.tile([P, GC * B_LOC], f32, tag="mapg")
                nc.vector.tensor_tensor(
                    out=mapg[:].rearrange("p (k s) -> p k s", s=B_LOC),
                    in0=Ag[:].rearrange("p (k s) -> p k s", s=B_LOC),
                    in1=r[:].to_broadcast([P, GC, B_LOC]),
                    op=mybir.AluOpType.mult,
                )
                for k in range(GC):
                    nc.tensor.matmul(
                        out=psum[:],
                        lhsT=mapg[:, k * B_LOC : (k + 1) * B_LOC],
                        rhs=Wg[:, k * E : (k + 1) * E],
                        start=(g == 0 and k == 0),
                        stop=(g == NG - 1 and k == GC - 1),
                    )

            # bag rows out (v = V-1)
            oB = smpool.tile([B_LOC, E], f32, tag="oB", bufs=1)
            nc.vector.tensor_copy(oB[:], psum[:])
            out_sv = out_d[:].rearrange("(s v) e -> s v e", v=V)
            nc.sync.dma_start(out=out_sv[:, V - 1, :], in_=oB[:])

            # ---------------- singles renorm + store (tail) ----------------
            sqS = spool.tile([P, NJ * E], f32, tag="sqS", bufs=1)
            nc.vector.tensor_mul(sqS[:], gS[:], gS[:])
            n2S = spool.tile([P, NJ], f32, tag="n2S", bufs=1)
            nc.vector.tensor_reduce(
                n2S[:], sqS[:].rearrange("p (j e) -> p j e", e=E),
                axis=mybir.AxisListType.X, op=mybir.AluOpType.add,
            )
            nc.vector.tensor_scalar_max(n2S[:], n2S[:], 1.0)
            sq2S = spool.tile([P, NJ], f32, tag="sq2S", bufs=1)
            nc.scalar.activation(
                sq2S[:], n2S[:], mybir.ActivationFunctionType.Sqrt,
                bias=zbias[:],
            )
            rS = spool.tile([P, NJ], f32, tag="rS", bufs=1)
            nc.vector.reciprocal(rS[:], sq2S[:])
            nc.vector.tensor_mul(rS[:], rS[:], wS_t[:])
            oS = spool.tile([P, NJ * E], f32, tag="oS", bufs=1)
            nc.vector.tensor_tensor(
                out=oS[:].rearrange("p (j e) -> p j e", e=E),
                in0=gS[:].rearrange("p (j e) -> p j e", e=E),
                in1=rS[:].to_broadcast([P, NJ, E]),
                op=mybir.AluOpType.mult,
            )
            # stream position i = j*128 + p = v*32 + s; p = 32q + s, v = 4j + q.
            # One DMA per q: partitions [32q, 32q+32) hold rows v = 4j + q.
            out48 = out_sv[:, 0 : V - 2, :].rearrange(
                "s (j q) e -> q s j e", q=4
            )
            for q in range(4):
                nc.sync.dma_start(
                    out=out48[q],
                    in_=oS[32 * q : 32 * (q + 1), :].rearrange(
                        "s (j e) -> s j e", e=E
                    )[:, 0:12, :],
                )
            nc.sync.dma_start(
                out=out_sv[:, V - 2, :],
                in_=oS[0:32, 12 * E : 13 * E],
            )

    nc.compile()
    return nc


def build_in_maps(inputs):
    """Host prep: returns (nc, in_maps, prep) for the SPMD run."""
    W = np.ascontiguousarray(np.asarray(inputs["W"], dtype=np.float32))
    codes_in = np.asarray(inputs["codes"])
    nv = np.asarray(inputs["n_visits"]).astype(np.int64)

    B, V, C = codes_in.shape
    VOCAB, E = W.shape
    L = V * C
    codes = np.ascontiguousarray(codes_in.reshape(B, L).astype(np.int32))

    prep = _prepare(codes, nv, B, V, C, L, VOCAB, E)
    nc = _build(V, E, prep["B_LOC"])
    Wp = permute_W(W, VOCAB, E)
    A16 = prep["A"].astype(ml_dtypes.bfloat16)
    in_maps = [
        {
            "W": Wp,
            "A": A16[k],
            "idxS": prep["idxS"][k],
            "wS": prep["wS"][k],
        }
        for k in range(N_CORES)
    ]
    return nc, in_maps, prep


def kernel(**inputs) -> np.ndarray:
    global LAST_RESULTS
    B, V, C = np.asarray(inputs["codes"]).shape
    E = np.asarray(inputs["W"]).shape[1]
    nc, in_maps, prep = build_in_maps(inputs)

    trace = bool(int(os.environ.get("KERNEL_TRACE", "0")))
    res = run_bass_kernel_spmd(
        nc, in_maps, core_ids=list(range(N_CORES)), trace=trace
    )
    LAST_RESULTS = res

    B_LOC = prep["B_LOC"]
    full = np.zeros((B, V, E), np.float32)
    for k in range(N_CORES):
        full[k * B_LOC : (k + 1) * B_LOC] = res.results[k]["out"].reshape(
            B_LOC, V, E
        )
    return full
